# revision 22
# baseline (speedup 1.0000x reference)
"""Trainium2 Bass kernel for nn_BlockRC3 (PRM dilated-conv stem + Token_performer block).

Contract: kernel(**inputs) takes FULL unsharded inputs (x [4,65536,64] fp32 + weights),
returns FULL output [4,16384,320] fp32. Internally: data-parallel over 8 NeuronCores,
each core handles half an image (8192 output tokens); the only cross-core communication
is a pair-wise AllReduce of the performer attention global sums (kptv [320,160] + ks [160]).

Layout strategy per core:
  - conv computed as weight-stationary matmuls over "parity plane" input tiles with
    K=128 tap-pair packing; output t lands channel-major [320c, 8192t] (3 chunks).
  - stage-1: one big fused matmul per 128-token tile: lhsT = t-chunk [c,128t],
    rhs = Wbig [c, 1346] producing token-major [128t, v|wtxk|wtxq|Zk|Zq|mu|Et2].
    LN1 mean is folded into weights on host; LN1 rstd applied as per-partition
    ACT scale on eviction; performer exp via single ACT Exp(scale=rstd, bias=-xd-ln(M)/2).
  - kptv/ks accumulated over tiles with lhsT = v-tile (+ones), rhs = kp-tile; AllReduce.
  - phase-2 channel-major: fused (kptv@proj) x qp matmul, LN2 via stats matmuls and
    host-folded mean, per-token scalars replicated across partitions by DMA.
"""

import math
import os

import numpy as np
import ml_dtypes

import concourse.bacc as bacc
import concourse.mybir as mybir
import concourse.tile as tile
from concourse.bass_utils import run_bass_kernel_spmd

FP32 = mybir.dt.float32
BF16 = mybir.dt.bfloat16
AF = mybir.ActivationFunctionType
ALU = mybir.AluOpType

B, N_IN, CIN = 4, 65536, 64
H = W = 256
EMB, ED, M = 320, 160, 160
T_CORE = 8192           # tokens per core (half image: 64 out rows x 128 cols)
NTILE = 64              # 128-token tiles per core
NBLK = 16               # 512-token blocks per core
PR, PC = 66, 130        # parity plane rows/cols (from padded 131x260 input)
PLANE = PR * PC
EPS_LN = 1e-5
EPS_ATTN = 1e-8
LNM_HALF = 0.5 * math.log(M)

# stage-1 psum column layout (3 banks: 512 + 512 + 322)
# bank0: v 0:320 | wtxk 320:480 | pad
# bank1: wtxq 0:160 | Zk 160:480 | pad
# bank2: Zq 0:320 | mu 320 | Et2 321
WBIG_COLS = 1346

# stored channel permutation: stored[j] = original channel perm[j]
PERM = np.concatenate([
    np.arange(0, 128),          # conv1 o 0:128
    np.arange(160, 288),        # conv2 o 0:128
    np.arange(128, 160),        # conv1 o 128:160
    np.arange(288, 320),        # conv2 o 128:160
])

_BF = ml_dtypes.bfloat16


def _bf16(a):
    return np.ascontiguousarray(a, dtype=np.float32).astype(_BF)


def _shift_flat(plane, delta):
    """plane [64, PR*PC]; return copy shifted left by delta (zero pad tail)."""
    out = np.zeros_like(plane)
    out[:, : PLANE - delta] = plane[:, delta:]
    return out


def host_prepare_weights(inp):
    """All host-side folds. Returns dict of numpy arrays to feed the kernel."""
    g1 = inp["ln1_g"].astype(np.float64)
    b1 = inp["ln1_b"].astype(np.float64)
    kqv_w = inp["kqv_w"].astype(np.float64)          # [960, 320]
    kqv_b = inp["kqv_b"].astype(np.float64)
    pw = inp["perf_w"].astype(np.float64)            # [160, 320]

    Wp = kqv_w * g1[None, :]                         # LN gain fold
    b_fold = kqv_b + kqv_w @ b1                      # LN bias fold
    s = Wp.sum(axis=1)
    Wpp = Wp - s[:, None] / EMB                      # LN mean fold
    Wk, Wq, Wv = Wpp[0:EMB], Wpp[EMB:2 * EMB], Wpp[2 * EMB:]
    bk, bq, bv = b_fold[0:EMB], b_fold[EMB:2 * EMB], b_fold[2 * EMB:]
    PK = pw @ Wk                                     # [160m, 320c] (contract e)
    PQ = pw @ Wq
    # post-exp constant vectors if biases nonzero (not used on the fast path)
    bias_zero = (np.abs(b_fold).max() == 0.0)

    # channel-permute the c axis of everything contracting with t
    p = PERM
    Wk_s, Wq_s, Wv_s = Wk[:, p], Wq[:, p], Wv[:, p]
    PK_s, PQ_s = PK[:, p], PQ[:, p]

    wbig = np.zeros((EMB, WBIG_COLS), np.float64)
    wbig[:, 0:320] = Wv_s.T
    wbig[:, 320:480] = PK_s.T
    wbig[:, 512:672] = PQ_s.T
    wbig[:, 672:992] = Wk_s.T
    wbig[:, 1024:1344] = Wq_s.T
    wbig[:, 1344] = 1.0   # mu column: device scales by 1/EMB (exact in bf16)
    # col 1345 left zero: Et2 accumulated by the t^2 matmul

    # conv weights: tap-pair stationary tiles.
    w1 = inp["conv_w1"].astype(np.float64)           # [160, 64, 3, 3]
    w2 = inp["conv_w2"].astype(np.float64)

    def tapw(w, dy, dx, osl):
        return np.ascontiguousarray(w[osl, :, dy, dx].T)   # [64, M]

    def pairw(w, tapA, tapB, osl):
        return np.concatenate([tapw(w, *tapA, osl), tapw(w, *tapB, osl)], axis=0)  # [128, M]

    lo, hi = slice(0, 128), slice(128, 160)
    conv = {}
    # conv1 pairs: (stack, base_flat, tapA, tapB)
    c1_pairs = [
        ("S3", 0,   (0, 0), (0, 2)),
        ("S3", 130, (2, 0), (2, 2)),
        ("S4", 1,   (0, 1), (2, 1)),
        ("S5", 130, (1, 0), (1, 2)),
    ]
    c1_single = ("S1u", 131, (1, 1))    # ee upper half of S1
    c2_pairs = [
        ("S1", 0,   (0, 0), (0, 1)),
        ("S1", 130, (1, 0), (1, 1)),
        ("S1", 260, (2, 0), (2, 1)),
        ("S2", 2,   (0, 2), (1, 2)),
    ]
    c2_single = ("S1l", 261, (2, 2))    # ee<<1 lower half of S1
    for osl, tag in ((lo, "lo"), (hi, "hi")):
        conv[f"c1_{tag}_pairs"] = [(st, off, _bf16(pairw(w1, tA, tB, osl)))
                                   for st, off, tA, tB in c1_pairs]
        conv[f"c1_{tag}_single"] = (c1_single[0], c1_single[1],
                                    _bf16(tapw(w1, *c1_single[2], osl)))
        conv[f"c2_{tag}_pairs"] = [(st, off, _bf16(pairw(w2, tA, tB, osl)))
                                   for st, off, tA, tB in c2_pairs]
        conv[f"c2_{tag}_single"] = (c2_single[0], c2_single[1],
                                    _bf16(tapw(w2, *c2_single[2], osl)))

    # conv bias per stored channel [320]
    cb = np.concatenate([inp["conv_b1"], inp["conv_b2"]]).astype(np.float64)[PERM]

    # LN2 + MLP folds
    g2 = inp["ln2_g"].astype(np.float64)
    b2 = inp["ln2_b"].astype(np.float64)
    w_1 = inp["mlp_w1"].astype(np.float64)           # [320, 320]
    b_1 = inp["mlp_b1"].astype(np.float64)
    w_2 = inp["mlp_w2"].astype(np.float64)
    b_2 = inp["mlp_b2"].astype(np.float64)
    V1 = w_1 * g2[None, :]
    b1f = b_1 + w_1 @ b2
    s1 = V1.sum(axis=1)
    V1pp = V1 - s1[:, None] / EMB                    # [320h, 320e]
    proj_w = inp["proj_w"].astype(np.float64)        # [320, 320]
    proj_b = inp["proj_b"].astype(np.float64)

    out = dict(
        wbig=_bf16(wbig),
        conv=conv,
        ident=np.eye(128, dtype=np.float32).astype(_BF),
        conv_bias=cb.astype(np.float32),
        bias_zero=bias_zero,
        bk=bk.astype(np.float32), bq=bq.astype(np.float32), bv=bv.astype(np.float32),
        pw=pw.astype(np.float32),
        # phase-2 weights (channel-major contractions; e/h axes raw order,
        # but e axis of V1pp/proj must be in STORED order? no: phase-2 operates on
        # ya2 whose channel order is the ORIGINAL order (see below) -> keep raw.
        V1pp=_bf16(V1pp.T),                          # [320e, 320h] as lhsT chunks
        b1f=b1f.astype(np.float32),
        W2T=_bf16(w_2.T),                            # [320h, 320e] as lhsT chunks
        b_2=b_2.astype(np.float32),
        projwT=_bf16(proj_w.T),                      # [320e, 320o] lhsT for PKV fold
        proj_b=proj_b.astype(np.float32),
    )
    return out


def host_prepare_core_input(x, core):
    """x [B, 65536, 64] fp32 -> per-core stacked parity planes (bf16)."""
    b, half = core // 2, core % 2
    xi = np.ascontiguousarray(x[b].reshape(H, W, CIN).transpose(2, 0, 1))  # [64,256,256]
    r0 = 128 * half - 2
    pad = np.zeros((CIN, 131, 260), np.float32)
    rlo, rhi = max(r0, 0), min(r0 + 131, H)
    pad[:, rlo - r0:rhi - r0, 2:258] = xi[:, rlo:rhi, :]
    ee = pad[:, 0::2, 0::2]                      # [64, 66, 130]
    eo = pad[:, 0::2, 1::2]
    oe = np.zeros((CIN, PR, PC), np.float32); oe[:, :65] = pad[:, 1::2, 0::2]
    oo = np.zeros((CIN, PR, PC), np.float32); oo[:, :65] = pad[:, 1::2, 1::2]
    ee = ee.reshape(CIN, PLANE); eo = eo.reshape(CIN, PLANE)
    oe = oe.reshape(CIN, PLANE); oo = oo.reshape(CIN, PLANE)
    stacks = {
        "S1": np.concatenate([ee, _shift_flat(ee, 1)], axis=0),
        "S2": np.concatenate([ee, _shift_flat(ee, 130)], axis=0),
        "S3": np.concatenate([oo, _shift_flat(oo, 1)], axis=0),
        "S4": np.concatenate([oe, _shift_flat(oe, 130)], axis=0),
        "S5": np.concatenate([eo, _shift_flat(eo, 1)], axis=0),
    }
    return {k: _bf16(v) for k, v in stacks.items()}


# ---------------------------------------------------------------------------
# device kernel builder
# ---------------------------------------------------------------------------

def build_nc():
    nc = bacc.Bacc(None, target_bir_lowering=False)

    def din(name, shape, dt=BF16):
        return nc.declare_dram_parameter(name, list(shape), dt, isOutput=False)

    # per-core inputs
    stacks_ext = {s: din(f"stk_{s}", [128, PLANE]) for s in ("S1", "S2", "S3", "S4", "S5")}
    # shared weights
    wbig_ext = [din("wbig0", [128, WBIG_COLS]), din("wbig1", [128, WBIG_COLS]),
                din("wbig2", [64, WBIG_COLS])]
    cw_ext = {}
    for cv in ("c1", "c2"):
        cw_ext[f"{cv}_lo_pairs"] = din(f"{cv}_lo_pairs", [128, 4 * 128])
        cw_ext[f"{cv}_hi_pairs"] = din(f"{cv}_hi_pairs", [128, 4 * 32])
    cw_ext["singles_lo"] = din("singles_lo", [128, 128])   # rows 0:64 c1(1,1), 64:128 c2(2,2)
    cw_ext["singles_hi"] = din("singles_hi", [128, 32])
    convb_ext = din("convb", [320], FP32)
    v1_ext = din("v1pp", [320, 320])
    w2t_ext = din("w2t", [320, 320])
    pjt_ext = din("projwT", [320, 320])
    b1f_ext = din("b1f", [320], FP32)
    b2_ext = din("b2v", [320], FP32)
    pjb_ext = din("projb", [320], FP32)
    ident_ext = din("ident", [128, 128])

    out_ext = nc.declare_dram_parameter("out", [320, T_CORE], BF16, isOutput=True)

    ECH = [(0, 128), (128, 128), (256, 64)]     # channel chunks (offset, size)
    MCH = [(0, 128), (128, 32)]                 # m (performer feature) chunks

    with tile.TileContext(nc) as tc:
        import contextlib
        with contextlib.ExitStack() as ctx:
            persist = ctx.enter_context(tc.tile_pool(name="persist", bufs=1))
            dram = ctx.enter_context(tc.tile_pool(name="dram", bufs=1, space="DRAM"))

            # ---- persistent small constants ----
            eps_ln_t = persist.tile([128, 1], FP32)
            nc.vector.memset(eps_ln_t, EPS_LN)
            ones_ks = persist.tile([128, 1], BF16)
            nc.vector.memset(ones_ks, 1.0)
            ones_t2 = [persist.tile([p, 1], FP32, name=f"ones_t2_{i}")
                       for i, (_, p) in enumerate(ECH)]
            for t in ones_t2:
                nc.vector.memset(t, 1.0 / EMB)
            ones_e = [persist.tile([p, 1], BF16, name=f"ones_e_{i}")
                      for i, (_, p) in enumerate(ECH)]
            for t in ones_e:
                nc.vector.memset(t, 1.0)
            convb_sb = persist.tile([128, 3], FP32)      # col0: c1lo, col1: c2lo, col2: hi(64)
            nc.sync.dma_start(out=convb_sb[:, 0:1], in_=convb_ext[0:128].rearrange("(b one) -> b one", one=1))
            nc.sync.dma_start(out=convb_sb[:, 1:2], in_=convb_ext[128:256].rearrange("(b one) -> b one", one=1))
            nc.sync.dma_start(out=convb_sb[0:64, 2:3], in_=convb_ext[256:320].rearrange("(b one) -> b one", one=1))
            b1f_sb = persist.tile([128, 3], FP32)
            for i, (o, p) in enumerate(ECH):
                nc.sync.dma_start(out=b1f_sb[0:p, i:i + 1], in_=b1f_ext[o:o + p].rearrange("(b one) -> b one", one=1))
            b2_sb = persist.tile([128, 3], FP32)
            for i, (o, p) in enumerate(ECH):
                nc.sync.dma_start(out=b2_sb[0:p, i:i + 1], in_=b2_ext[o:o + p].rearrange("(b one) -> b one", one=1))
            pjb_sb = persist.tile([128, 3], FP32)
            for i, (o, p) in enumerate(ECH):
                nc.sync.dma_start(out=pjb_sb[0:p, i:i + 1], in_=pjb_ext[o:o + p].rearrange("(b one) -> b one", one=1))
            ident_sb = persist.tile([128, 128], BF16)
            nc.sync.dma_start(out=ident_sb, in_=ident_ext[:, :])
            ones_row = persist.tile([1, 128], BF16)
            nc.vector.memset(ones_row, 1.0)

            # DRAM bounce buffers
            cc_in = dram.tile([321, M], FP32)
            cc_out = dram.tile([321, M], FP32)

            # =================== PHASES A+B scope (t channel-major lives here) ==========
            ab_stack = ctx.enter_context(contextlib.ExitStack())
            pab = ab_stack.enter_context(tc.tile_pool(name="pab", bufs=1))
            tc_sb = [pab.tile([p, T_CORE], BF16, name=f"tc_sb_{i}")
                     for i, (_, p) in enumerate(ECH)]

            # =================== PHASE A: conv ===================
            with tc.tile_pool(name="convp", bufs=1) as convp, \
                 tc.tile_pool(name="cpsum", bufs=2, space="PSUM") as cpsum:
                stk = {}
                for s, ext in stacks_ext.items():
                    t = convp.tile([128, PLANE], BF16, name=f"stk_{s}_sb")
                    nc.sync.dma_start(out=t, in_=ext[:, :])
                    stk[s] = t
                cw = {}
                for k, ext in cw_ext.items():
                    t = convp.tile([128, ext.shape[1]], BF16, name=f"cw_{k}_sb")
                    nc.sync.dma_start(out=t, in_=ext[:, :])
                    cw[k] = t

                def stack_view(name):
                    base = stk[name[:2]]
                    r = base.rearrange("p (r c) -> p r c", c=PC)
                    if name.endswith("u"):
                        return r[0:64]
                    if name.endswith("l"):
                        return r[64:128]
                    return r

                def conv_rhs(stname, flat_off, blk):
                    ro, co = divmod(flat_off, PC)
                    v = stack_view(stname)
                    h0 = blk * 4
                    return v[:, h0 + ro:h0 + ro + 4, co:co + 128]

                PAIR_DEFS = {
                    "c1": ([("S3", 0), ("S3", 130), ("S4", 1), ("S5", 130)], ("S1u", 131)),
                    "c2": ([("S1", 0), ("S1", 130), ("S1", 260), ("S2", 2)], ("S1l", 261)),
                }

                for blk in range(NBLK):
                    ps_lo1 = cpsum.tile([128, 512], FP32, tag="pslo1")
                    ps_lo2 = cpsum.tile([128, 512], FP32, tag="pslo2")
                    ps_hi = cpsum.tile([64, 512], FP32, tag="pshi")
                    for cvi, cv in enumerate(("c1", "c2")):
                        pairs, single = PAIR_DEFS[cv]
                        ps = (ps_lo1, ps_lo2)[cvi]
                        wlo = cw[f"{cv}_lo_pairs"]
                        whi = cw[f"{cv}_hi_pairs"]
                        for k, (st, off) in enumerate(pairs):
                            rhs = conv_rhs(st, off, blk)
                            nc.tensor.matmul(ps, wlo[:, k * 128:(k + 1) * 128], rhs,
                                             start=(k == 0), stop=False)
                            nc.tensor.matmul(ps_hi[cvi * 32:(cvi + 1) * 32, :],
                                             whi[:, k * 32:(k + 1) * 32], rhs,
                                             start=(k == 0), stop=False,
                                             tile_position=(0, 32 * cvi))
                        st, off = single
                        rhs = conv_rhs(st, off, blk)
                        wsl = cw["singles_lo"][cvi * 64:(cvi + 1) * 64, :]
                        wsh = cw["singles_hi"][cvi * 64:(cvi + 1) * 64, :]
                        nc.tensor.matmul(ps, wsl, rhs, start=False, stop=True,
                                         tile_position=(64 * cvi, 0))
                        nc.tensor.matmul(ps_hi[cvi * 32:(cvi + 1) * 32, :], wsh, rhs,
                                         start=False, stop=True,
                                         tile_position=(64 * cvi, 32 * cvi))
                    csl = slice(blk * 512, (blk + 1) * 512)
                    nc.scalar.activation(out=tc_sb[0][:, csl], in_=ps_lo1,
                                         func=AF.Gelu, bias=convb_sb[:, 0:1])
                    nc.scalar.activation(out=tc_sb[1][:, csl], in_=ps_lo2,
                                         func=AF.Gelu, bias=convb_sb[:, 1:2])
                    nc.scalar.activation(out=tc_sb[2][:, csl], in_=ps_hi,
                                         func=AF.Gelu, bias=convb_sb[0:64, 2:3])

            # channel-major homes for v / qp (consumed by phase C; filled by
            # the PE-transpose pass at the end of phase B). qp chunk1 (m
            # 128:160) lives in v_cm[2] partitions 64:96.
            bcp = ctx.enter_context(tc.tile_pool(name="bcp", bufs=1))
            v_cm = [bcp.tile([128, T_CORE], BF16, name=f"v_cm_{i}")
                    for i in range(3)]
            qp_cm0 = bcp.tile([128, T_CORE], BF16, name="qp_cm0")

            # =================== PHASE B: stage-1 + kptv ===================
            with tc.tile_pool(name="pb", bufs=1) as pb:
                v_sb = pb.tile([128, NTILE, 320], BF16, name="v_sb")
                qp_sb = pb.tile([128, NTILE, 160], BF16, name="qp_sb")
                wbig_sb = []
                for i, ext in enumerate(wbig_ext):
                    t = pb.tile([ext.shape[0], WBIG_COLS], BF16, name=f"wbig_sb_{i}")
                    nc.sync.dma_start(out=t, in_=ext[:, :])
                    wbig_sb.append(t)
                stA = pb.tile([128, 480], FP32, name="stA")
                stB = pb.tile([1, 160], FP32, name="stB")

                with tc.tile_pool(name="spsum", bufs=6, space="PSUM") as spsum, \
                     tc.tile_pool(name="kpsum", bufs=1, space="PSUM") as kpsum, \
                     tc.tile_pool(name="bwork", bufs=4) as bwork:
                    psA = kpsum.tile([128, 480], FP32)
                    psB = kpsum.tile([1, 160], FP32)

                    for i in range(NTILE):
                        tsl = slice(i * 128, (i + 1) * 128)
                        b0 = spsum.tile([128, 512], FP32, tag="s1", name="b0")
                        b1 = spsum.tile([128, 512], FP32, tag="s1", name="b1")
                        b2 = spsum.tile([128, 512], FP32, tag="s1", name="b2")
                        for kc in range(3):
                            lhsT = tc_sb[kc][:, tsl]
                            nc.tensor.matmul(b0, lhsT, wbig_sb[kc][:, 0:512],
                                             start=(kc == 0), stop=(kc == 2))
                            nc.tensor.matmul(b1, lhsT, wbig_sb[kc][:, 512:1024],
                                             start=(kc == 0), stop=(kc == 2))
                            nc.tensor.matmul(b2[:, 0:322], lhsT, wbig_sb[kc][:, 1024:1346],
                                             start=(kc == 0), stop=False)
                        for kc in range(3):
                            _, pch = ECH[kc]
                            t2 = bwork.tile([128, 128], FP32, tag="t2", name="t2")
                            nc.vector.tensor_tensor(out=t2[0:pch, :], in0=tc_sb[kc][:, tsl],
                                                    in1=tc_sb[kc][:, tsl], op=ALU.mult)
                            nc.tensor.matmul(b2[:, 321:322], t2[0:pch, :], ones_t2[kc],
                                             start=False, stop=(kc == 2))

                        # per-token LN1 stats (Et2 column already scaled by 1/EMB)
                        mu_s = bwork.tile([128, 1], FP32, tag="sc", bufs=14, name="mu_s")
                        nc.vector.tensor_scalar(out=mu_s, in0=b2[:, 320:321],
                                                scalar1=1.0 / EMB, scalar2=None, op0=ALU.mult)
                        musq = bwork.tile([128, 1], FP32, tag="sc", bufs=14, name="musq")
                        nc.vector.tensor_tensor(out=musq, in0=mu_s, in1=mu_s, op=ALU.mult)
                        var_t = bwork.tile([128, 1], FP32, tag="sc", bufs=14, name="var_t")
                        nc.vector.tensor_tensor(out=var_t, in0=b2[:, 321:322], in1=musq,
                                                op=ALU.subtract)
                        std_t = bwork.tile([128, 1], FP32, tag="sc", bufs=14, name="std_t")
                        nc.scalar.activation(out=std_t, in_=var_t, func=AF.Sqrt,
                                             bias=eps_ln_t)
                        rstd_t = bwork.tile([128, 1], FP32, tag="sc", bufs=14, name="rstd_t")
                        nc.vector.reciprocal(out=rstd_t, in_=std_t)

                        # |k|^2, |q|^2 with rstd^2 folded in via the ACT scale
                        scr = bwork.tile([128, 320], FP32, tag="scr", name="scr")
                        ssk2 = bwork.tile([128, 1], FP32, tag="sc", bufs=14, name="ssk2")
                        nc.scalar.activation(out=scr, in_=b1[:, 160:480], func=AF.Square,
                                             scale=rstd_t, accum_out=ssk2)
                        scr2 = bwork.tile([128, 320], FP32, tag="scr", name="scr2")
                        ssq2 = bwork.tile([128, 1], FP32, tag="sc", bufs=14, name="ssq2")
                        nc.scalar.activation(out=scr2, in_=b2[:, 0:320], func=AF.Square,
                                             scale=rstd_t, accum_out=ssq2)
                        bk2_t = bwork.tile([128, 1], FP32, tag="sc", bufs=14, name="bk2_t")
                        nc.vector.tensor_scalar(out=bk2_t, in0=ssk2, scalar1=-0.5,
                                                scalar2=-LNM_HALF, op0=ALU.mult, op1=ALU.add)
                        bq2_t = bwork.tile([128, 1], FP32, tag="sc", bufs=14, name="bq2_t")
                        nc.vector.tensor_scalar(out=bq2_t, in0=ssq2, scalar1=-0.5,
                                                scalar2=-LNM_HALF, op0=ALU.mult, op1=ALU.add)

                        # evictions
                        kp_t = bwork.tile([128, 160], BF16, tag="kpt", bufs=4, name="kp_t")
                        nc.scalar.activation(out=kp_t, in_=b0[:, 320:480],
                                             func=AF.Exp, bias=bk2_t, scale=rstd_t)
                        nc.scalar.activation(out=qp_sb[:, i, :], in_=b1[:, 0:160],
                                             func=AF.Exp, bias=bq2_t, scale=rstd_t)
                        nc.scalar.activation(out=v_sb[:, i, :], in_=b0[:, 0:320],
                                             func=AF.Identity, scale=rstd_t)

                        # kptv/ks accumulation
                        first, last = (i == 0), (i == NTILE - 1)
                        nc.tensor.matmul(psA[:, 0:160], v_sb[:, i, 0:128], kp_t,
                                         start=first, stop=last)
                        nc.tensor.matmul(psA[:, 160:320], v_sb[:, i, 128:256], kp_t,
                                         start=False, stop=last, skip_group_check=True)
                        nc.tensor.matmul(psA[0:64, 320:480], v_sb[:, i, 256:320], kp_t,
                                         start=False, stop=last, skip_group_check=True)
                        nc.tensor.matmul(psB, ones_ks, kp_t,
                                         start=first, stop=last)

                    # ship partial sums to DRAM and all-reduce with the pair core
                    nc.vector.tensor_copy(out=stA[:, 0:320], in_=psA[:, 0:320])
                    nc.vector.tensor_copy(out=stA[0:64, 320:480], in_=psA[0:64, 320:480])
                    nc.vector.tensor_copy(out=stB, in_=psB)

                nc.sync.dma_start(out=cc_in[0:128, :], in_=stA[:, 0:160])
                nc.sync.dma_start(out=cc_in[128:256, :], in_=stA[:, 160:320])
                nc.sync.dma_start(out=cc_in[256:320, :], in_=stA[0:64, 320:480])
                nc.sync.dma_start(out=cc_in[320:321, :], in_=stB)
                nc.gpsimd.collective_compute(
                    "AllReduce", ALU.add,
                    replica_groups=[[0, 1], [2, 3], [4, 5], [6, 7]],
                    ins=[cc_in.opt()], outs=[cc_out.opt()],
                )

                # PE-transpose pass: v/qp token-major -> channel-major in SBUF.
                # Overlaps the AllReduce; phase C only needs cc_out + v_cm/qp_cm.
                with tc.tile_pool(name="tpsum", bufs=2, space="PSUM") as tpsum:
                    for i in range(NTILE):
                        tsl = slice(i * 128, (i + 1) * 128)
                        psTa = tpsum.tile([128, 512], BF16, tag="tp", bufs=4, name="psTa")
                        psTb = tpsum.tile([128, 128], BF16, tag="tpb", bufs=2, name="psTb")
                        nc.tensor.matmul(psTa[:, 0:128], v_sb[:, i, 0:128], ident_sb,
                                         is_transpose=True, start=True, stop=True)
                        nc.tensor.matmul(psTa[:, 128:256], v_sb[:, i, 128:256], ident_sb,
                                         is_transpose=True, start=True, stop=True,
                                         skip_group_check=True)
                        nc.tensor.matmul(psTa[0:64, 256:384], v_sb[:, i, 256:320], ident_sb,
                                         is_transpose=True, start=True, stop=True,
                                         skip_group_check=True)
                        nc.tensor.matmul(psTa[:, 384:512], qp_sb[:, i, 0:128], ident_sb,
                                         is_transpose=True, start=True, stop=True,
                                         skip_group_check=True)
                        nc.tensor.matmul(psTb[64:96, 0:128], qp_sb[:, i, 128:160], ident_sb,
                                         is_transpose=True, start=True, stop=True,
                                         tile_position=(0, 64))
                        nc.scalar.activation(out=v_cm[0][:, tsl], in_=psTa[:, 0:128],
                                             func=AF.Identity)
                        nc.scalar.activation(out=v_cm[1][:, tsl], in_=psTa[:, 128:256],
                                             func=AF.Identity)
                        nc.vector.tensor_copy(out=v_cm[2][0:64, tsl], in_=psTa[0:64, 256:384])
                        nc.vector.tensor_copy(out=qp_cm0[:, tsl], in_=psTa[:, 384:512])
                        nc.vector.tensor_copy(out=v_cm[2][64:96, tsl], in_=psTb[64:96, 0:128])

            # =================== PHASE C: attention tail + MLP ===================
            with tc.tile_pool(name="pcp", bufs=1) as pcp, \
                 tc.tile_pool(name="cwork", bufs=3) as cwork, \
                 tc.tile_pool(name="cpsum2", bufs=3, space="PSUM") as cps, \
                 tc.tile_pool(name="rpsum", bufs=2, space="PSUM") as rps:
                # phase-2 weights
                def load_chunks(ext, width, nm):
                    tiles = []
                    for i, (o, p) in enumerate(ECH):
                        t = pcp.tile([p, width], BF16, name=f"{nm}_{i}")
                        nc.sync.dma_start(out=t, in_=ext[o:o + p, :])
                        tiles.append(t)
                    return tiles

                v1_sb = load_chunks(v1_ext, 320, "v1_sb")
                w2t_sb = load_chunks(w2t_ext, 320, "w2t_sb")
                pjt_sb = load_chunks(pjt_ext, 320, "pjt_sb")

                # collective results -> SBUF (fp32) -> bf16
                kpe16 = []
                for i, (o, p) in enumerate(ECH):
                    tf = cwork.tile([p, 160], FP32, tag="kpef", name="kpef")
                    nc.sync.dma_start(out=tf, in_=cc_out[o:o + p, :])
                    tb = pcp.tile([p, 160], BF16, name=f"kpe16_{i}")
                    nc.vector.tensor_copy(out=tb, in_=tf)
                    kpe16.append(tb)
                # ks as per-partition columns; chunk1 (m 128:160) sits at partitions 64:96
                ks_col = []
                for mi, (mo, mp) in enumerate(MCH):
                    pbase = 0 if mi == 0 else 64
                    tf = cwork.tile([128, 1], FP32, tag="ksf", name="ksf")
                    nc.sync.dma_start(out=tf[pbase:pbase + mp, :],
                                      in_=cc_out[320:321, mo:mo + mp].rearrange("a b -> b a"))
                    tb = pcp.tile([128, 1], BF16, name=f"ks16_{mo}")
                    nc.vector.tensor_copy(out=tb[pbase:pbase + mp, :],
                                          in_=tf[pbase:pbase + mp, :])
                    ks_col.append((tb, pbase))

                # PKV = kptv.T @ proj_w.T  [m, 320o], then bf16.
                # chunk1 lands at partitions 64:96 to align with qp chunk1's home.
                pkv_sb = []
                for mi, (mo, mp) in enumerate(MCH):
                    pbase = 0 if mi == 0 else 64
                    psPKV = cps.tile([128, 512], FP32, tag="big", name="psPKV")
                    for ec in range(3):
                        nc.tensor.matmul(psPKV[pbase:pbase + mp, 0:320],
                                         kpe16[ec][:, mo:mo + mp], pjt_sb[ec],
                                         start=(ec == 0), stop=(ec == 2),
                                         tile_position=(0, pbase))
                    tb = pcp.tile([128, 320], BF16, name=f"pkv_sb_{mi}")
                    nc.vector.tensor_copy(out=tb[pbase:pbase + mp, :],
                                          in_=psPKV[pbase:pbase + mp, 0:320])
                    pkv_sb.append((tb, pbase))

                qp_views = [(qp_cm0, 0), (v_cm[2], 64)]

                # block loop
                for blk in range(NBLK):
                    bsl = slice(blk * 512, (blk + 1) * 512)
                    # D = ks . qp  -> [1, 512]
                    psD = rps.tile([1, 512], FP32, tag="row", bufs=3, name="psD")
                    for mi in range(2):
                        mp = MCH[mi][1]
                        qt, qo = qp_views[mi]
                        kt, ko = ks_col[mi]
                        nc.tensor.matmul(psD, kt[ko:ko + mp, :], qt[qo:qo + mp, bsl],
                                         start=(mi == 0), stop=(mi == 1))
                    dinv_row = cwork.tile([1, 512], FP32, tag="drow", bufs=8, name="dinv_row")
                    nc.vector.tensor_scalar(out=dinv_row, in0=psD, scalar1=EPS_ATTN,
                                            scalar2=None, op0=ALU.add)
                    nc.vector.reciprocal(out=dinv_row, in_=dinv_row)
                    dinv_bf = cwork.tile([1, 512], BF16, tag="drow", bufs=8, name="dinv_bf")
                    nc.vector.tensor_copy(out=dinv_bf, in_=dinv_row)
                    # broadcast to 128 partitions via ones-matmul
                    psDrep = rps.tile([128, 512], FP32, tag="rep", bufs=2, name="psDrep")
                    nc.tensor.matmul(psDrep, ones_row, dinv_bf, start=True, stop=True)
                    dinv_rep = cwork.tile([128, 512], BF16, tag="rep", name="dinv_rep")
                    nc.vector.tensor_copy(out=dinv_rep, in_=psDrep)

                    # fused attn+proj: psP = PKV.T @ qp
                    ya2 = []
                    ya2sq = []
                    for oc, (o, p) in enumerate(ECH):
                        psP = cps.tile([128, 512], FP32, tag="big", name="psP")
                        for mi in range(2):
                            mp = MCH[mi][1]
                            qt, qo = qp_views[mi]
                            pt, po = pkv_sb[mi]
                            nc.tensor.matmul(psP[0:p, :], pt[po:po + mp, o:o + p],
                                             qt[qo:qo + mp, bsl],
                                             start=(mi == 0), stop=(mi == 1))
                        tmp = cwork.tile([128, 512], FP32, tag="tmp", name="tmp")
                        nc.vector.tensor_tensor(out=tmp[0:p, :], in0=psP[0:p, :],
                                                in1=dinv_rep[0:p, :], op=ALU.mult)
                        nc.vector.tensor_scalar(out=tmp[0:p, :], in0=tmp[0:p, :],
                                                scalar1=pjb_sb[0:p, oc:oc + 1], scalar2=None,
                                                op0=ALU.add)
                        y2 = cwork.tile([128, 512], BF16, tag="ya2", name="y2")
                        nc.vector.tensor_tensor(out=y2[0:p, :], in0=tmp[0:p, :],
                                                in1=v_cm[oc][0:p, bsl], op=ALU.add)
                        ya2.append(y2)
                        ysq = cwork.tile([128, 512], BF16, tag="ysq", name="ysq")
                        nc.vector.tensor_tensor(out=ysq[0:p, :], in0=y2[0:p, :],
                                                in1=y2[0:p, :], op=ALU.mult)
                        ya2sq.append(ysq)

                    # LN2 stats
                    psMu = rps.tile([1, 512], FP32, tag="row", bufs=3, name="psMu")
                    psS2 = rps.tile([1, 512], FP32, tag="row", bufs=3, name="psS2")
                    for ec, (o, p) in enumerate(ECH):
                        nc.tensor.matmul(psMu, ones_e[ec], ya2[ec][0:p, :],
                                         start=(ec == 0), stop=(ec == 2))
                        nc.tensor.matmul(psS2, ones_e[ec], ya2sq[ec][0:p, :],
                                         start=(ec == 0), stop=(ec == 2))
                    mu2 = cwork.tile([1, 512], FP32, tag="drow", bufs=8, name="mu2")
                    nc.vector.tensor_scalar(out=mu2, in0=psMu, scalar1=1.0 / EMB,
                                            scalar2=None, op0=ALU.mult)
                    s2m = cwork.tile([1, 512], FP32, tag="drow", bufs=8, name="s2m")
                    nc.vector.tensor_scalar(out=s2m, in0=psS2, scalar1=1.0 / EMB,
                                            scalar2=None, op0=ALU.mult)
                    musq2 = cwork.tile([1, 512], FP32, tag="drow", bufs=8, name="musq2")
                    nc.vector.tensor_tensor(out=musq2, in0=mu2, in1=mu2, op=ALU.mult)
                    var2 = cwork.tile([1, 512], FP32, tag="drow", bufs=8, name="var2")
                    nc.vector.tensor_tensor(out=var2, in0=s2m, in1=musq2, op=ALU.subtract)
                    std2 = cwork.tile([1, 512], FP32, tag="drow", bufs=8, name="std2")
                    nc.scalar.activation(out=std2, in_=var2, func=AF.Sqrt,
                                         bias=eps_ln_t[0:1, :])
                    rstd2_row = cwork.tile([1, 512], FP32, tag="drow", bufs=8, name="rstd2_row")
                    nc.vector.reciprocal(out=rstd2_row, in_=std2)
                    rstd2_bf = cwork.tile([1, 512], BF16, tag="drow", bufs=8, name="rstd2_bf")
                    nc.vector.tensor_copy(out=rstd2_bf, in_=rstd2_row)
                    psRrep = rps.tile([128, 512], FP32, tag="rep", bufs=2, name="psRrep")
                    nc.tensor.matmul(psRrep, ones_row, rstd2_bf, start=True, stop=True)
                    rstd2_rep = cwork.tile([128, 512], BF16, tag="rep", name="rstd2_rep")
                    nc.vector.tensor_copy(out=rstd2_rep, in_=psRrep)

                    # mlp1 + gelu
                    g_sb = []
                    for hc, (ho, hp) in enumerate(ECH):
                        psH = cps.tile([128, 512], FP32, tag="big", name="psH")
                        for ec, (o, p) in enumerate(ECH):
                            nc.tensor.matmul(psH[0:hp, :], v1_sb[ec][:, ho:ho + hp],
                                             ya2[ec][0:p, :],
                                             start=(ec == 0), stop=(ec == 2))
                        tmp = cwork.tile([128, 512], FP32, tag="tmp", name="tmpH")
                        nc.vector.tensor_tensor(out=tmp[0:hp, :], in0=psH[0:hp, :],
                                                in1=rstd2_rep[0:hp, :], op=ALU.mult)
                        g = cwork.tile([128, 512], BF16, tag="gsb", name="g")
                        nc.scalar.activation(out=g[0:hp, :], in_=tmp[0:hp, :],
                                             func=AF.Gelu, bias=b1f_sb[0:hp, hc:hc + 1])
                        g_sb.append(g)

                    # mlp2 + skip + store
                    for oc, (o, p) in enumerate(ECH):
                        psO = cps.tile([128, 512], FP32, tag="big", name="psO")
                        for hc, (ho, hp) in enumerate(ECH):
                            nc.tensor.matmul(psO[0:p, :], w2t_sb[hc][:, o:o + p],
                                             g_sb[hc][0:hp, :],
                                             start=(hc == 0), stop=(hc == 2))
                        outt = cwork.tile([128, 512], FP32, tag="outt", name="outt")
                        nc.vector.tensor_scalar(out=outt[0:p, :], in0=psO[0:p, :],
                                                scalar1=b2_sb[0:p, oc:oc + 1], scalar2=None,
                                                op0=ALU.add)
                        outb = cwork.tile([128, 512], BF16, tag="outb", name="outb")
                        nc.vector.tensor_tensor(out=outb[0:p, :], in0=outt[0:p, :],
                                                in1=ya2[oc][0:p, :], op=ALU.add)
                        nc.sync.dma_start(out=out_ext[o:o + p, bsl], in_=outb[0:p, :])

    nc.finalize()
    return nc


# ---------------------------------------------------------------------------
# host entry
# ---------------------------------------------------------------------------

_NC_CACHE = {}


def _get_nc():
    if "nc" not in _NC_CACHE:
        _NC_CACHE["nc"] = build_nc()
    return _NC_CACHE["nc"]


def _numpy_reference(inp):
    """Fallback path (only taken for nonzero kqv/ln1 bias, never in practice)."""
    from scipy.special import erf as _erf

    x = inp["x"].astype(np.float32)
    Bn, Nn, Cn = x.shape
    Hn = Wn = int(round(math.sqrt(Nn)))
    xi = x.transpose(0, 2, 1).reshape(Bn, Cn, Hn, Wn)

    def conv(xw, w, b, dil, pad):
        xp = np.pad(xw, ((0, 0), (0, 0), (pad, pad), (pad, pad)))
        Ho = Wo = Hn // 2
        cols = np.empty((Bn, Cn * 9, Ho * Wo), np.float32)
        i = 0
        for dy in range(3):
            for dx in range(3):
                sl = xp[:, :, dy * dil:dy * dil + 2 * Ho:2, dx * dil:dx * dil + 2 * Wo:2]
                cols[:, i * Cn:(i + 1) * Cn, :] = sl.reshape(Bn, Cn, -1)
                i += 1
        wm = w.transpose(0, 2, 3, 1).reshape(ED, 9 * Cn)
        return (wm[None] @ cols + b[None, :, None]).reshape(Bn, ED, Ho, Wo)

    def gelu(t):
        return t * 0.5 * (1 + _erf(t / np.sqrt(2.0)))

    y1 = gelu(conv(xi, inp["conv_w1"], inp["conv_b1"], 1, 1))
    y2 = gelu(conv(xi, inp["conv_w2"], inp["conv_b2"], 2, 2))
    y = np.concatenate([y1, y2], 1)
    t = y.reshape(Bn, EMB, -1).transpose(0, 2, 1)

    def ln(z, g, b):
        mu = z.mean(-1, keepdims=True)
        var = z.var(-1)[..., None]
        return (z - mu) / np.sqrt(var + EPS_LN) * g + b

    h = ln(t, inp["ln1_g"], inp["ln1_b"])
    kqv = h @ inp["kqv_w"].T + inp["kqv_b"]
    k, q, v = kqv[..., :EMB], kqv[..., EMB:2 * EMB], kqv[..., 2 * EMB:]
    pwm = inp["perf_w"]

    def prm(z):
        xd = 0.5 * (z * z).sum(-1, keepdims=True)
        return np.exp(z @ pwm.T - xd) / math.sqrt(M)

    kp, qp = prm(k), prm(q)
    D = np.matmul(qp, kp.sum(1)[..., None])
    kptv = np.matmul(v.transpose(0, 2, 1), kp)
    ya = np.matmul(qp, kptv.transpose(0, 2, 1)) / (D + EPS_ATTN)
    ya = v + (ya @ inp["proj_w"].T + inp["proj_b"])
    h2 = ln(ya, inp["ln2_g"], inp["ln2_b"])
    g = gelu(h2 @ inp["mlp_w1"].T + inp["mlp_b1"])
    return (ya + (g @ inp["mlp_w2"].T + inp["mlp_b2"])).astype(np.float32)


def prepare_in_maps(inp):
    prep = host_prepare_weights(inp)
    if not prep["bias_zero"]:
        return None

    shared = {
        "wbig0": prep["wbig"][0:128], "wbig1": prep["wbig"][128:256],
        "wbig2": prep["wbig"][256:320],
        "convb": prep["conv_bias"].reshape(320),
        "v1pp": prep["V1pp"], "w2t": prep["W2T"], "projwT": prep["projwT"],
        "b1f": prep["b1f"].reshape(320), "b2v": prep["b_2"].reshape(320),
        "projb": prep["proj_b"].reshape(320),
        "ident": prep["ident"],
    }
    conv = prep["conv"]
    for cv in ("c1", "c2"):
        shared[f"{cv}_lo_pairs"] = np.concatenate(
            [w for _, _, w in conv[f"{cv}_lo_pairs"]], axis=1)
        shared[f"{cv}_hi_pairs"] = np.concatenate(
            [w for _, _, w in conv[f"{cv}_hi_pairs"]], axis=1)
    s_lo = np.concatenate([conv["c1_lo_single"][2], conv["c2_lo_single"][2]], axis=0)
    s_hi = np.concatenate([conv["c1_hi_single"][2], conv["c2_hi_single"][2]], axis=0)
    shared["singles_lo"] = s_lo
    shared["singles_hi"] = s_hi

    in_maps = []
    for core in range(8):
        stacks = host_prepare_core_input(inp["x"], core)
        m = dict(shared)
        for s, arr in stacks.items():
            m[f"stk_{s}"] = arr
        in_maps.append(m)
    return in_maps


def kernel(**inputs):
    inp = {k: np.asarray(v) for k, v in inputs.items()}
    in_maps = prepare_in_maps(inp)
    if in_maps is None:
        return _numpy_reference(inp)

    nc = _get_nc()
    res = run_bass_kernel_spmd(nc, in_maps, list(range(8)))
    _NC_CACHE["last_results"] = res
    _NC_CACHE["last_in_maps"] = in_maps
    out = np.empty((B, 16384, EMB), np.float32)
    for core in range(8):
        b, half = core // 2, core % 2
        out[b, half * T_CORE:(half + 1) * T_CORE, :] = \
            res.results[core]["out"].astype(np.float32).T
    return out

